# revision 10
# baseline (speedup 1.0000x reference)
"""Trainium2 Bass kernel for Grid-attention-linear.

Math (per batch element, B=8 sharded 1/core over 8 cores):
  g_up   = bilinear_resize(g, 64x64 -> 256x256)          [128, 256, 256]
  g_conv = W @ g_up + b                                  [64, 256, 256]
  att    = sum_c g_conv^2                                [256, 256]
  att    = (att - min(att)) / sum(att - min(att))
  out[c] = sum_n x[c, n] * att[n]                        [64]

Key identity used on-device: resize and 1x1-conv commute, and
  sum_c (up(gc))^2 = sum_k  Ay_k @ C_k @ Ax_k^T
where gc = W@g + b at low res, C_k are 5 channel-summed shifted
self-correlation maps of gc (shifts (0,0),(0,1),(1,0),(1,1),(1,-1)),
and Ay_k/Ax_k are small per-phase separable upsample-weight matrices
derived from the exact bilinear resize matrix R[256, 64].

Device outputs per core: att_mm = att_raw - min (f32 [256,256]) and
u = x . att_mm (f32 [1,64]).  Host divides both by S = att_mm.sum().
"""

import numpy as np

import concourse.bass as bass
import concourse.mybir as mybir
from concourse.tile import TileContext
from concourse.bass_utils import run_bass_kernel_spmd

F32 = mybir.dt.float32
AL = mybir.AluOpType
AF = mybir.ActivationFunctionType
AX = mybir.AxisListType

C_IN = 128     # g channels
C_OUT = 64     # x / conv-out channels
HW_LO = 64     # low-res spatial
N_LO = HW_LO * HW_LO          # 4096
HW_HI = 256
PAD = 128                     # free-dim padding after gc for shifted reads
N_CORES = 8


# ---------------------------------------------------------------------------
# host-side constant construction
# ---------------------------------------------------------------------------

def _resize_mat(n_in, n_out):
    """Exact linear operator of jax.image.resize(..., method='bilinear')
    (half-pixel centers, triangle kernel, edge weight renormalization)."""
    A = np.zeros((n_out, n_in), np.float64)
    scale = n_in / n_out
    for i in range(n_out):
        xs = (i + 0.5) * scale - 0.5
        x0 = int(np.floor(xs))
        f = xs - x0
        for j, w in ((x0, 1.0 - f), (x0 + 1, f)):
            if 0 <= j < n_in:
                A[i, j] += w
    A /= A.sum(axis=1, keepdims=True)
    return A


def _host_consts():
    R = _resize_mat(HW_LO, HW_HI)            # [256, 64]
    Ay0 = R * R                              # weights for same-y pairs
    Ay1 = np.zeros_like(R)
    Ay1[:, :-1] = 2.0 * R[:, :-1] * R[:, 1:]  # (y, y+1) pairs
    Ax0 = Ay0
    Ax1 = Ay1
    Ax1h = Ay1 * 0.5                          # diagonal pairs carry the 2 once

    def f32c(a):
        return np.ascontiguousarray(a, dtype=np.float32)

    c = {}
    # AxT_k: [64 x-part, 256 jx]
    c["axt0"] = f32c(Ax0.T)
    c["axt1"] = f32c(Ax1.T)
    c["axt1h"] = f32c(Ax1h.T)
    # AyT parity split: [32 y2, 256 iy]
    for v, Ay in ((0, Ay0), (1, Ay1)):
        AyT = Ay.T                            # [64 y, 256 iy]
        c[f"ayte{v}"] = f32c(AyT[0::2, :])
        c[f"ayto{v}"] = f32c(AyT[1::2, :])
    c["ident"] = f32c(np.eye(128))
    return c


# ---------------------------------------------------------------------------
# device program
# ---------------------------------------------------------------------------

def _legalize_waits(nc, max_waits=1):
    """walrus in this env rejects instructions with >1 sync wait; split the
    extras onto same-engine NoOps placed immediately before."""
    n = 0
    for f in nc.m.functions:
        for blk in f.blocks:
            out = []
            for inst in blk.instructions:
                si = getattr(inst, "sync_info", None)
                if si is not None and si.on_wait and len(si.on_wait) > max_waits:
                    extra = si.on_wait[:-max_waits]
                    keep = list(si.on_wait[-max_waits:])
                    for w in extra:
                        nop = mybir.InstNoOp(
                            name=f"waitfix-{n}",
                            engine=inst.engine,
                            ins=[],
                            outs=[],
                            sync_info=mybir.SyncInfo(on_wait=[w], on_update=[]),
                        )
                        n += 1
                        out.append(nop)
                    inst.sync_info = mybir.SyncInfo(
                        on_wait=keep, on_update=si.on_update)
                out.append(inst)
            blk.instructions[:] = out
    return n


def _build_program():
    nc = bass.Bass("TRN2")

    x_h = nc.dram_tensor("x", [C_OUT, 2, 128, HW_HI], F32, kind="ExternalInput")
    g_h = nc.dram_tensor("g", [C_IN, N_LO], F32, kind="ExternalInput")
    wt_h = nc.dram_tensor("wt", [C_IN, C_OUT], F32, kind="ExternalInput")
    b_h = nc.dram_tensor("bvec", [C_OUT, 1], F32, kind="ExternalInput")
    axt_h = {
        k: nc.dram_tensor(k, [HW_LO, HW_HI], F32, kind="ExternalInput")
        for k in ("axt0", "axt1", "axt1h")
    }
    ayt_h = {
        k: nc.dram_tensor(k, [32, HW_HI], F32, kind="ExternalInput")
        for k in ("ayte0", "ayto0", "ayte1", "ayto1")
    }
    ident_h = nc.dram_tensor("ident", [128, 128], F32, kind="ExternalInput")

    att_out = nc.dram_tensor("att_out", [HW_HI, HW_HI], F32, kind="ExternalOutput")
    u_out = nc.dram_tensor("u_out", [1, C_OUT], F32, kind="ExternalOutput")

    with TileContext(nc) as tc:
        with (
            tc.tile_pool(name="singles", bufs=1) as singles,
            tc.tile_pool(name="gpool", bufs=1) as gpool,
            tc.tile_pool(name="ppool", bufs=2) as ppool,
            tc.tile_pool(name="xpool", bufs=20) as xpool,
            tc.tile_pool(name="spool", bufs=2) as spool,
            tc.tile_pool(name="psA", bufs=2, space="PSUM") as psA,
            tc.tile_pool(name="psB", bufs=2, space="PSUM") as psB,
        ):
            # ---- constants into SBUF
            wt_sb = singles.tile_from(wt_h[:, :])
            b_sb = singles.tile_from(b_h[:, :])
            axt_sb = {k: singles.tile_from(h[:, :], name=k) for k, h in axt_h.items()}
            ayt_sb = {k: singles.tile_from(h[:, :], name=k) for k, h in ayt_h.items()}
            ident_sb = singles.tile_from(ident_h[:, :])

            ones64 = singles.tile([C_OUT, 1], F32, tag="ones64")
            nc.any.memset(ones64[:, :], 1.0)
            ones1r = singles.tile([1, 128], F32, tag="ones1r")
            nc.any.memset(ones1r[:, :], 1.0)
            onesp = singles.tile([128, 1], F32, tag="onesp")
            nc.any.memset(onesp[:, :], 1.0)

            # ---- load g, conv -> gc (with bias), padded for shifted reads
            g_sb = gpool.tile([C_IN, N_LO], F32, tag="g")
            nc.sync.dma_start(g_sb[:, :], g_h[:, :])

            gc_sb = gpool.tile([C_OUT, N_LO + PAD], F32, tag="gc")
            nc.any.memset(gc_sb[:, N_LO:], 0.0)
            # absorb the wt DMA wait on PE before the first real matmul so no
            # single matmul needs two DMA-queue waits (walrus S3_LW limit)
            warm_ps = psA.tile([C_OUT, 1], F32, name="warmps", tag="ps")
            nc.tensor.matmul(warm_ps[:, :], wt_sb[:, :], wt_sb[:, 0:1],
                             start=True, stop=True)
            for j in range(N_LO // 512):
                gc_ps = psA.tile([C_OUT, 512], F32, tag="ps")
                nc.tensor.matmul(
                    gc_ps[:, :], wt_sb[:, :], g_sb[:, j * 512:(j + 1) * 512],
                    start=True, stop=True,
                )
                nc.scalar.add(gc_sb[:, j * 512:(j + 1) * 512], gc_ps[:, :],
                              add=b_sb[:, 0:1])

            # ---- att accumulation PSUM tiles (iy-chunk major)
            att_ps = [psB.tile([128, HW_HI], F32, name=f"att{ic}", tag=f"att{ic}") for ic in (0, 1)]

            # 5 correlation maps: (shift0, shift1, AxT const, Ay variant)
            map_specs = [
                (None, None, "axt0", 0),      # C00 = gc*gc        (ActE square)
                (0, 1, "axt1", 0),            # C01
                (0, HW_LO, "axt0", 1),        # C10
                (0, HW_LO + 1, "axt1h", 1),   # C11a
                (HW_LO, 1, "axt1h", 1),       # C11b
            ]

            n_maps = len(map_specs)
            for k, (s0, s1, axk, v) in enumerate(map_specs):
                p_t = ppool.tile([C_OUT, N_LO], F32, tag="prod")
                if s0 is None:
                    nc.scalar.activation(p_t[:, :], gc_sb[:, 0:N_LO], AF.Square)
                else:
                    nc.vector.tensor_mul(
                        p_t[:, :],
                        gc_sb[:, s0:s0 + N_LO],
                        gc_sb[:, s1:s1 + N_LO],
                    )

                # channel sum -> mapT2 [128=(y parity, x), 32=y-pair]
                mt_ps = psA.tile([128, 32], F32, tag="ps")
                for j in range(32):
                    nc.tensor.matmul(
                        mt_ps[:, j:j + 1],
                        p_t[:, j * 128:(j + 1) * 128],
                        ones64[:, :],
                        start=True, stop=True,
                    )
                mt_sb = spool.tile([128, 32], F32, tag="mt")
                nc.scalar.copy(mt_sb[:, :], mt_ps[:, :])
                # odd-parity half must sit at base partition 0 for matmul
                mt_o = spool.tile([64, 32], F32, tag="mto")
                nc.sync.dma_start(mt_o[:, :], mt_sb[64:128, :])

                # stage 1: M1{e,o}[y2, jx] = sum_x C[y, x] * Ax[jx, x]
                m1_sb = {}
                for par, mt_half in ((0, mt_sb[0:64, :]), (1, mt_o[:, :])):
                    m1_ps = psA.tile([32, HW_HI], F32, name=f"m1ps{par}", tag="ps")
                    nc.tensor.matmul(m1_ps[:, :], mt_half,
                                     axt_sb[axk][:, :], start=True, stop=True)
                    m1t = spool.tile([32, HW_HI], F32, name=f"m1sb{par}",
                                     tag=f"m1{par}")
                    nc.scalar.copy(m1t[:, :], m1_ps[:, :])
                    m1_sb[par] = m1t

                # stage 2: att[iy, jx] += sum_y Ay[iy, y] * M1[y, jx]
                for ic in (0, 1):
                    for par, ay_key in ((0, f"ayte{v}"), (1, f"ayto{v}")):
                        nc.tensor.matmul(
                            att_ps[ic][:, :],
                            ayt_sb[ay_key][:, ic * 128:(ic + 1) * 128],
                            m1_sb[par][:, :],
                            start=(k == 0 and par == 0),
                            stop=(k == n_maps - 1 and par == 1),
                            skip_group_check=True,
                        )

            # ---- global min of att_raw
            mn = [singles.tile([128, 1], F32, name=f"mn{ic}", tag=f"mn{ic}") for ic in (0, 1)]
            for ic in (0, 1):
                nc.vector.tensor_reduce(mn[ic][:, :], att_ps[ic][:, :],
                                        axis=AX.X, op=AL.min)
            mnc = singles.tile([128, 1], F32, tag="mnc")
            nc.vector.scalar_tensor_tensor(mnc[:, :], mn[0][:, :], 1.0,
                                           mn[1][:, :], op0=AL.mult, op1=AL.min)
            tr_ps = psA.tile([1, 128], F32, tag="ps")
            nc.tensor.transpose(tr_ps[:, :], mnc[:, :], ident_sb[:, :])
            mrow = singles.tile([1, 128], F32, tag="mrow")
            nc.scalar.copy(mrow[:, :], tr_ps[:, :])
            m_sb = singles.tile([1, 1], F32, tag="msb")
            nc.vector.tensor_reduce(m_sb[:, :], mrow[:, :], axis=AX.X, op=AL.min)

            bc_ps = psA.tile([128, 1], F32, tag="ps")
            nc.tensor.matmul(bc_ps[:, :], ones1r[:, :], m_sb[:, :],
                             start=True, stop=True)
            negm = singles.tile([128, 1], F32, tag="negm")
            nc.scalar.mul(negm[:, :], bc_ps[:, :], -1.0)

            # ---- att_sb = att_raw - min   [128, (ic, jx)]
            att_sb = singles.tile([128, 2 * HW_HI], F32, tag="attsb")
            for ic in (0, 1):
                nc.scalar.add(att_sb[:, ic * HW_HI:(ic + 1) * HW_HI],
                              att_ps[ic][:, :], add=negm[:, 0:1])
            nc.sync.dma_start(
                att_out.rearrange("(a p) w -> p a w", a=2),
                att_sb.rearrange("p (a w) -> p a w", a=2),
            )

            # ---- u[c] = sum_n x[c, n] * att_sb[n]
            uacc = singles.tile([128, C_OUT], F32, tag="uacc")
            for c in range(C_OUT):
                x_t = xpool.tile([128, 2 * HW_HI], F32, tag="xt")
                nc.sync.dma_start(
                    x_t.rearrange("p (a w) -> p a w", a=2),
                    x_h[c].rearrange("a p w -> p a w"),
                )
                scr = spool.tile([128, 2 * HW_HI], F32, tag="scr")
                nc.vector.scalar_tensor_tensor(
                    out=scr[:, :],
                    in0=x_t[:, :],
                    scalar=1.0,
                    in1=att_sb[:, :],
                    op0=AL.mult,
                    op1=AL.mult,
                    accum_out=uacc[:, c:c + 1],
                )

            u_ps = psA.tile([1, C_OUT], F32, tag="ps")
            nc.tensor.matmul(u_ps[:, :], onesp[:, :], uacc[:, :],
                             start=True, stop=True)
            u_sb = singles.tile([1, C_OUT], F32, tag="usb")
            nc.scalar.copy(u_sb[:, :], u_ps[:, :])
            nc.sync.dma_start(u_out[:, :], u_sb[:, :])

    _legalize_waits(nc)
    return nc


_CACHE = {}


def _get_program():
    if "nc" not in _CACHE:
        _CACHE["nc"] = _build_program()
        _CACHE["consts"] = _host_consts()
    return _CACHE["nc"], _CACHE["consts"]


def kernel(x, g, W, b):
    x = np.asarray(x, dtype=np.float32)
    g = np.asarray(g, dtype=np.float32)
    W = np.asarray(W, dtype=np.float32)
    b = np.asarray(b, dtype=np.float32)
    B = x.shape[0]
    assert B == N_CORES

    nc, consts = _get_program()
    wt = np.ascontiguousarray(W.T)                       # [128, 64]
    bvec = np.ascontiguousarray(b.reshape(C_OUT, 1))

    in_maps = []
    for i in range(B):
        m = {
            "x": np.ascontiguousarray(
                x[i].reshape(C_OUT, 2, 128, HW_HI)),
            "g": np.ascontiguousarray(g[i].reshape(C_IN, N_LO)),
            "wt": wt,
            "bvec": bvec,
        }
        m.update(consts)
        in_maps.append(m)

    res = run_bass_kernel_spmd(nc, in_maps, core_ids=list(range(N_CORES)))

    out = np.zeros((B, C_OUT), np.float32)
    att = np.zeros((B, HW_HI, HW_HI), np.float32)
    for i in range(B):
        att_mm = res.results[i]["att_out"]
        u = res.results[i]["u_out"].reshape(C_OUT)
        S = np.float32(att_mm.sum(dtype=np.float32))
        out[i] = u / S
        att[i] = att_mm / S
    return out, att
